# revision 41
# baseline (speedup 1.0000x reference)
"""Trainium2 Bass kernel for LAES linear recurrence + deep readout (v4).

Math: h_t = (x_t - bias) @ A.T + h_{t-1} @ B.T  (T=512 steps, h0=0),
then out = tanh(tanh(h@W1.T+b1)@W2.T+b2)@W3.T+b3.

Design (v1 37.5us -> v2 36.5 -> v3 35.2 -> v4 32.5 -> v6 ~31.1us typ):
1. Whole pre-tanh pipeline is linear in x: Y = sum_g D_g @ (x_{T-1-g}-bias),
   D_g = W1 B^g A (host fp64 weight precompute).  Main lags g < Km=10
   stream in fp8-e3m4 (scale target 8 -> 1.3% per-entry rms error).
2. Linearized corrections (weights-only; c1,c2 calibrated on synthetic
   gaussian x): every approximation error E (lag quant, W2 quant,
   truncated tail lags 10..21) maps to output space as a [*,10] matrix
   G ~ c1*c2*E.T@W2.T@W3.T, accumulated on-device by tiny matmuls that
   reuse already-loaded PE weights.  This lets W2 stream in fp8-e3m4
   (1MB instead of 2MB) and truncates the recurrence at 10 exact lags.
3. Biases enter PSUM via K=1 matmuls (ones row x bias row), so phase
   outputs stay in [batch, hidden] layout and evacuate with plain tanh
   in two [64,512] activations per phase (PSUM banks are 2KB/partition).
   Both phases run a SPLIT TAIL: the last lags/k-tiles accumulate into
   the first PSUM bank only, closing it ~1.3us early so its tanh and the
   first PE transposes overlap the second bank's remaining matmuls.
4. Layout flips Z1/Z2 [64b,1024h] -> 8x[128h,64b] use PE transposes
   interleaved with their consumer matmuls (T_k ... k-matmuls), with DVE
   evacuating each transposed tile PSUM->SBUF.
5. Total HBM stream ~2.75MB/core across both HWDGE rings + SWDGE,
   chunked in consumption order so phase 1 starts ~11us in and W2
   overlaps phase 1.  PE warm-up matmuls (throwaway groups in a real
   PSUM bank) ramp the clock during the initial DMA fill.
   Data-parallel over batch: 64 cols/core, no collectives.
"""

import sys

for _p in ("/opt/trn_rl_repo", "/root/.axon_site/_ro/trn_rl_repo"):
    if _p not in sys.path:
        sys.path.append(_p)

import numpy as np
import ml_dtypes

import concourse.bass as bass  # noqa: F401  (bass must import before bacc)
import concourse.mybir as mybir
import concourse.tile as tile
from concourse import bacc
from concourse.bass_utils import run_bass_kernel_spmd

T, BATCH, IN, HID, NCLS = 512, 512, 128, 1024, 10
NCORES = 8
SB = BATCH // NCORES   # batch columns per core
Km = 9                 # exact lags (all fp8-e3m4)
Kc = 22                # corrected lags (tail handled via Gt only)
NT = HID // 128
HH = HID // 2          # 512: psum bank width (fp32)

F32 = mybir.dt.float32
F16 = mybir.dt.float16
F8E3 = mybir.dt.float8e3
F8E4 = mybir.dt.float8e4
NPE3 = ml_dtypes.float8_e3m4
NPE4 = ml_dtypes.float8_e4m3fn
ACT = mybir.ActivationFunctionType

# f16 blob column offsets
XH_O = 0
XH_W = Km * SB                 # 576
W3_O = XH_O + XH_W
W3_W = NT * NCLS               # 80
GT_O = W3_O + W3_W
GT_W = (Kc - Km) * NCLS        # 130
GQ_O = GT_O + GT_W
GQ_W = Km * NCLS               # 90
G1_O = GQ_O + GQ_W
G1_W = NT * NCLS               # 80
ID_O = G1_O + G1_W
F16W = ID_O + 64

# brow (single-partition f16) offsets
B1_O = 0
B2_O = HID
ON_O = 2 * HID
B3R_O = ON_O + SB
BROWW = B3R_O + 16             # 2128

E2 = 32.0                      # W2 power-2 scale (asserted vs host)
ECORR = 256.0                  # correction-column power-2 scale (asserted)

_PROGRAM_CACHE = {}


def _build_program(ncores=NCORES):
    nc = bacc.Bacc(
        "TRN2",
        target_bir_lowering=False,
        debug=False,
        num_devices=ncores,
    )

    F16Bd = nc.dram_tensor("F16B", [128, F16W], F16, kind="ExternalInput").ap()
    BROWd = nc.dram_tensor("BROW", [1, BROWW], F16, kind="ExternalInput").ap()
    D8d = nc.dram_tensor("D8", [128, Km, HID], F8E3, kind="ExternalInput").ap()
    W2d = nc.dram_tensor("W2P", [128, NT, HID], F8E3, kind="ExternalInput").ap()
    XCd = nc.dram_tensor("XC", [128, (Kc - Km) * SB], F8E4, kind="ExternalInput").ap()
    outd = nc.dram_tensor("out", [NCLS, SB], F32, kind="ExternalOutput").ap()

    with tile.TileContext(nc) as tc:
        with (
            tc.tile_pool(name="cst", bufs=1) as cp,
            tc.tile_pool(name="sb", bufs=1) as sp,
            tc.tile_pool(name="psum", bufs=1, space="PSUM") as pp,
        ):
            # ---- SBUF tiles ----
            f16b = cp.tile([128, F16W], F16, tag="f16b")
            brow = cp.tile([1, BROWW], F16, tag="brow")
            d8 = cp.tile([128, Km, HID], F8E3, tag="d8")
            w2 = cp.tile([128, NT, HID], F8E3, tag="w2")
            xc = cp.tile([128, (Kc - Km), SB], F8E4, tag="xc")
            warm = cp.tile([128, HH], F16, tag="warm")
            yt = sp.tile([64, HID], F16, tag="yt")
            yt2 = sp.tile([64, HID], F16, tag="yt2")
            z1t = sp.tile([128, NT, SB], F16, tag="z1t")
            z2t = sp.tile([128, NT, SB], F16, tag="z2t")
            corrall = sp.tile([64, NCLS], F16, tag="corrall")
            ptCsb = sp.tile([NCLS, SB], F16, tag="ptCsb")
            ot = sp.tile([NCLS, SB], F32, tag="ot")

            # ---- DMA issue (order per ring = transfer order).  The
            # scalar HWDGE ring is empirically the fastest; it carries
            # the bulk in consumption order.
            nc.sync.dma_start(f16b[:], F16Bd[:])
            nc.scalar.dma_start(d8[:, 0:3, :], D8d[:, 0:3, :])    # lags 0-2
            nc.scalar.dma_start(d8[:, 3:5, :], D8d[:, 3:5, :])    # lags 3-4
            nc.scalar.dma_start(d8[:, 5:7, :], D8d[:, 5:7, :])    # lags 5-6
            nc.sync.dma_start(d8[:, 7:9, :], D8d[:, 7:9, :])      # lags 7-8
            nc.gpsimd.dma_start(brow[:], BROWd[:])
            nc.gpsimd.dma_start(xc[:], XCd[:])
            nc.scalar.dma_start(w2[:, 0:2, :], W2d[:, 0:2, :])
            nc.scalar.dma_start(w2[:, 2:4, :], W2d[:, 2:4, :])
            nc.scalar.dma_start(w2[:, 4:6, :], W2d[:, 4:6, :])
            nc.scalar.dma_start(w2[:, 6:8, :], W2d[:, 6:8, :])

            # ---- PSUM layout: psA/psB [64,512] (phase 1), psC/psD
            # (phase 2, same 2 slots), psG [64,10] corr cols, psO [10,64],
            # pt pool 2x [128,64] transposes -> 8 banks total.
            psA = pp.tile([64, HH], F32, tag="pA", bufs=1, name="psA")
            psB = pp.tile([64, HH], F32, tag="pB", bufs=1, name="psB")

            # ---- PE warm-up: throwaway groups in psB's bank ----
            nc.vector.memset(warm[:], 0.0)
            for r in range(10):
                n = HH if r < 7 else 128
                nc.tensor.matmul(
                    psB[:, 0:n], warm[:, 0:64], warm[:, 0:n],
                    start=(r == 0), stop=(r == 9),
                )

            # ---- phase 1: Y[64b, 1024h] over Km lags + Gq columns.
            # Lags 0..5 interleave psA/psB (data-paced); the tail lags run
            # psA-first so psA stops ~1.3us early and ACT-A + the first
            # transposes overlap the psB tail matmuls.
            psG = pp.tile([64, NCLS], F32, tag="psG", bufs=1)
            ones = brow[0:1, ON_O : ON_O + SB]
            # bias rows open the accumulation groups (brow arrives ~10.5us,
            # before the first lag chunk) - keeps them off the phase tail
            nc.tensor.matmul(
                psA[:], ones, brow[0:1, B1_O : B1_O + HH],
                start=True, stop=False,
            )
            nc.tensor.matmul(
                psB[:], ones, brow[0:1, B1_O + HH : B1_O + HID],
                start=True, stop=False,
            )
            HEAD1 = [0, 1, 2, 3, 4, 5]
            TAILL = [6, 7, 8]
            for gi, g in enumerate(HEAD1):
                xg = f16b[:, XH_O + g * SB : XH_O + (g + 1) * SB]
                nc.tensor.matmul(
                    psA[:], xg, d8[:, g, 0:HH],
                    start=False, stop=False,
                )
                nc.tensor.matmul(
                    psB[:], xg, d8[:, g, HH:HID],
                    start=False, stop=False,
                )
                nc.tensor.matmul(
                    psG[:], xg, f16b[:, GQ_O + g * NCLS : GQ_O + (g + 1) * NCLS],
                    start=(gi == 0), stop=False,
                )
            for gi, g in enumerate(TAILL):
                xg = f16b[:, XH_O + g * SB : XH_O + (g + 1) * SB]
                nc.tensor.matmul(
                    psA[:], xg, d8[:, g, 0:HH],
                    start=False, stop=(gi == len(TAILL) - 1),
                )
                nc.tensor.matmul(
                    psG[:], xg, f16b[:, GQ_O + g * NCLS : GQ_O + (g + 1) * NCLS],
                    start=False, stop=False,
                )
            for gi, g in enumerate(TAILL):
                xg = f16b[:, XH_O + g * SB : XH_O + (g + 1) * SB]
                nc.tensor.matmul(
                    psB[:], xg, d8[:, g, HH:HID],
                    start=False, stop=(gi == len(TAILL) - 1),
                )

            # ---- tail-lag corrections into psO [10, 64] ----
            psO = pp.tile([NCLS, SB], F32, tag="psO", bufs=1)
            for i in range(Kc - Km):
                nc.tensor.matmul(
                    psO[:],
                    f16b[:, GT_O + i * NCLS : GT_O + (i + 1) * NCLS],
                    xc[:, i, :],
                    start=(i == 0), stop=False,
                )

            # ---- evacuate phase 1: tanh -> yt (one ACT per bank) ----
            nc.scalar.activation(yt[:, 0:HH], psA[:], ACT.Tanh)
            nc.scalar.activation(yt[:, HH:HID], psB[:], ACT.Tanh)

            # ---- phase 2 with interleaved PE transposes of z1 tiles ----
            psC = pp.tile([64, HH], F32, tag="pA", bufs=1, name="psC")
            psD = pp.tile([64, HH], F32, tag="pB", bufs=1, name="psD")
            nc.tensor.matmul(
                psC[:], ones, brow[0:1, B2_O : B2_O + HH],
                start=True, stop=False,
            )
            nc.tensor.matmul(
                psD[:], ones, brow[0:1, B2_O + HH : B2_O + HID],
                start=True, stop=False,
            )

            def emit_T(zt, src_yt, k, nm):
                pt = pp.tile([128, SB], F16, tag="pt", bufs=2, name=nm)
                nc.tensor.transpose(
                    pt[:], src_yt[:, k * 128 : (k + 1) * 128],
                    f16b[0:64, ID_O : ID_O + 64],
                )
                nc.vector.tensor_copy(zt[:, k, :], pt[:])

            for k in range(4):
                emit_T(z1t, yt, k, f"pt{k}")
            TAIL2 = NT - 2
            for k in range(TAIL2):
                if k + 4 < NT:
                    emit_T(z1t, yt, k + 4, f"pt{k + 4}")
                zk = z1t[:, k, :]
                nc.tensor.matmul(
                    psC[:], zk, w2[:, k, 0:HH],
                    start=False, stop=False,
                )
                nc.tensor.matmul(
                    psD[:], zk, w2[:, k, HH:HID],
                    start=False, stop=False,
                )
                nc.tensor.matmul(
                    psG[:], zk, f16b[:, G1_O + k * NCLS : G1_O + (k + 1) * NCLS],
                    start=False, stop=False,
                )
            for k in range(TAIL2, NT):
                zk = z1t[:, k, :]
                nc.tensor.matmul(
                    psC[:], zk, w2[:, k, 0:HH],
                    start=False, stop=(k == NT - 1),
                )
                nc.tensor.matmul(
                    psG[:], zk, f16b[:, G1_O + k * NCLS : G1_O + (k + 1) * NCLS],
                    start=False, stop=(k == NT - 1),
                )
            for k in range(TAIL2, NT):
                zk = z1t[:, k, :]
                nc.tensor.matmul(
                    psD[:], zk, w2[:, k, HH:HID],
                    start=False, stop=(k == NT - 1),
                )
            

            # ---- evacuate phase 2: tanh(x/e2) -> yt2 ----
            nc.scalar.activation(yt2[:, 0:HH], psC[:], ACT.Tanh, scale=1.0 / E2)
            nc.scalar.activation(yt2[:, HH:HID], psD[:], ACT.Tanh, scale=1.0 / E2)

            # corr columns to fp16 while the out stage runs
            nc.vector.tensor_copy(corrall[:], psG[:])
            # b3 via K=1 matmul (independent of z2 - keep off the tail)
            nc.tensor.matmul(
                psO[:],
                brow[0:1, B3R_O : B3R_O + NCLS],
                ones,
                start=False, stop=False,
            )
            # transpose corr [64,10] -> [10,64] now (psG stopped long ago)
            ptC = pp.tile([128, SB], F16, tag="pt", bufs=2, name="ptC")
            nc.tensor.transpose(
                ptC[0:NCLS, :], corrall[:], f16b[0:64, ID_O : ID_O + 64]
            )
            nc.vector.tensor_copy(ptCsb[:], ptC[0:NCLS, :])

            # ---- out stage: psO += W3 @ z2, transposes interleaved ----
            emit_T(z2t, yt2, 0, "qt0")
            emit_T(z2t, yt2, 1, "qt1")
            for k in range(NT):
                if k + 2 < NT:
                    emit_T(z2t, yt2, k + 2, f"qt{k + 2}")
                nc.tensor.matmul(
                    psO[:],
                    f16b[:, W3_O + k * NCLS : W3_O + (k + 1) * NCLS],
                    z2t[:, k, :],
                    start=False, stop=(k == NT - 1),
                )
            # ot = ptCsb * (1/e_corr) + psO
            nc.vector.scalar_tensor_tensor(
                ot[:], ptCsb[:], 1.0 / ECORR, psO[:],
                mybir.AluOpType.mult, mybir.AluOpType.add,
            )
            nc.sync.dma_start(outd[:], ot[:])

    nc.compile()
    return nc


def _prep_weights(A, B, bias, W1, b1, W2, b2, W3, b3):
    """Host fp64 weight-only precompute (c1/c2 calibrated on synthetic
    gaussian x matching the spec'd input distribution, never the real x)."""
    B64 = B.astype(np.float64)
    W164 = W1.astype(np.float64)
    A64 = A.astype(np.float64)
    b64 = bias.astype(np.float64)
    W264 = W2.astype(np.float64)
    W364 = W3.astype(np.float64)

    Ds, M = [], A64.copy()
    for g in range(Kc):
        Ds.append(W164 @ M)
        M = B64 @ M
    Dsum = W164 @ np.linalg.solve(np.eye(HID) - B64, A64)
    b1f = b1.astype(np.float64) - Dsum @ b64

    rng = np.random.default_rng(12345)
    xcal = rng.standard_normal((Kc, 256, IN))
    Ycal = sum(xcal[g] @ Ds[g].T for g in range(Kc))
    c1 = float((1 - np.tanh(Ycal + b1f) ** 2).mean())
    y2cal = np.tanh(Ycal + b1f) @ W264.T + b2.astype(np.float64)
    c2 = float((1 - np.tanh(y2cal) ** 2).mean())

    D8 = np.empty((IN, Km, HID), NPE3)
    lagE, e_lag = [], []
    for g in range(Km):
        m = np.abs(Ds[g]).max()
        e = 2.0 ** np.clip(np.floor(np.log2(8.0 / m)), 0, 6)
        Dq = (Ds[g].T * e).astype(NPE3)
        D8[:, g, :] = Dq
        e_lag.append(e)
        lagE.append(e * Ds[g].T - Dq.astype(np.float64))

    mW2 = np.abs(W264).max()
    e2 = 2.0 ** np.floor(np.log2(8.0 / mW2))
    W2q = (W264.T * e2).astype(NPE3)              # [k, m]
    E2m = W264.T - W2q.astype(np.float64) / e2
    W2P = np.empty((IN, NT, HID), NPE3)
    for k in range(NT):
        W2P[:, k, :] = W2q[k * 128 : (k + 1) * 128, :]

    CWm = c1 * c2 * (W264.T @ W364.T)
    Gq = [lagE[g] @ CWm for g in range(Km)]       # [IN, 10] at xq scale
    G1 = c2 * (E2m @ W364.T)                      # [k, 10] applied to z1
    Gt = [Ds[g].T @ CWm for g in range(Km, Kc)]   # [IN, 10] at true x scale

    gmax = max(max(np.abs(g_).max() for g_ in Gq), np.abs(G1).max())
    e_corr = 2.0 ** np.floor(np.log2(8.0 / gmax))

    brow = np.zeros((1, BROWW), np.float16)
    brow[0, B1_O : B1_O + HID] = b1f.astype(np.float16)
    brow[0, B2_O : B2_O + HID] = (b2.astype(np.float64) * e2).astype(np.float16)
    brow[0, ON_O : ON_O + SB] = 1.0
    brow[0, B3R_O : B3R_O + NCLS] = b3.astype(np.float16)

    f16c = np.zeros((128, F16W), np.float16)
    W3T = W364.T.astype(np.float16)               # [HID, 10]
    for k in range(NT):
        f16c[:, W3_O + k * NCLS : W3_O + (k + 1) * NCLS] = (
            W3T[k * 128 : (k + 1) * 128, :]
        )
    for i in range(Kc - Km):
        f16c[:, GT_O + i * NCLS : GT_O + (i + 1) * NCLS] = Gt[i].astype(np.float16)
    for g in range(Km):
        f16c[:, GQ_O + g * NCLS : GQ_O + (g + 1) * NCLS] = (
            (Gq[g] * e_corr).astype(np.float16)
        )
    for k in range(NT):
        f16c[:, G1_O + k * NCLS : G1_O + (k + 1) * NCLS] = (
            (G1[k * 128 : (k + 1) * 128, :] * e_corr).astype(np.float16)
        )
    f16c[0:64, ID_O : ID_O + 64] = np.eye(64, dtype=np.float16)

    return {
        "e_lag": e_lag, "e2": e2, "e_corr": e_corr,
        "D8": D8, "W2P": W2P, "brow": brow, "f16c": f16c,
        "c1": c1, "c2": c2,
    }


def _prep_inputs(x, wp, ncores=NCORES):
    in_maps = []
    for c in range(ncores):
        bsl = slice(c * SB, (c + 1) * SB)
        f16b = wp["f16c"].copy()
        for g in range(Km):
            f16b[:, XH_O + g * SB : XH_O + (g + 1) * SB] = (
                x[T - 1 - g, bsl, :].T / wp["e_lag"][g]
            ).astype(np.float16)
        XC = np.empty((IN, (Kc - Km) * SB), NPE4)
        for i, g in enumerate(range(Km, Kc)):
            XC[:, i * SB : (i + 1) * SB] = x[T - 1 - g, bsl, :].T.astype(NPE4)
        in_maps.append(
            {
                "F16B": f16b,
                "BROW": wp["brow"],
                "D8": wp["D8"],
                "W2P": wp["W2P"],
                "XC": XC,
            }
        )
    return in_maps


def kernel(x, A, B, bias, W1, b1, W2, b2, W3, b3, _trace=False):
    wp = _prep_weights(A, B, bias, W1, b1, W2, b2, W3, b3)
    assert wp["e2"] == E2, "activation scale 1/e2 hardcoded in program"
    assert wp["e_corr"] == ECORR, "1/e_corr hardcoded in program"
    if "nc" not in _PROGRAM_CACHE:
        _PROGRAM_CACHE["nc"] = _build_program()
    nc = _PROGRAM_CACHE["nc"]
    in_maps = _prep_inputs(x, wp)
    res = run_bass_kernel_spmd(nc, in_maps, list(range(NCORES)), trace=_trace)
    _PROGRAM_CACHE["last_result"] = res
    out = np.empty((BATCH, NCLS), np.float32)
    for c in range(NCORES):
        out[c * SB : (c + 1) * SB, :] = res.results[c]["out"].T
    return out


# revision 42
# speedup vs baseline: 1.0668x; 1.0668x over previous
"""Trainium2 Bass kernel for LAES linear recurrence + deep readout (v4).

Math: h_t = (x_t - bias) @ A.T + h_{t-1} @ B.T  (T=512 steps, h0=0),
then out = tanh(tanh(h@W1.T+b1)@W2.T+b2)@W3.T+b3.

Design (v1 37.5us -> v2 36.5 -> v3 35.2 -> v4 32.5 -> v6 ~31.1us typ):
1. Whole pre-tanh pipeline is linear in x: Y = sum_g D_g @ (x_{T-1-g}-bias),
   D_g = W1 B^g A (host fp64 weight precompute).  Main lags g < Km=10
   stream in fp8-e3m4 (scale target 8 -> 1.3% per-entry rms error).
2. Linearized corrections (weights-only; c1,c2 calibrated on synthetic
   gaussian x): every approximation error E (lag quant, W2 quant,
   truncated tail lags 10..21) maps to output space as a [*,10] matrix
   G ~ c1*c2*E.T@W2.T@W3.T, accumulated on-device by tiny matmuls that
   reuse already-loaded PE weights.  This lets W2 stream in fp8-e3m4
   (1MB instead of 2MB) and truncates the recurrence at 10 exact lags.
3. Biases enter PSUM via K=1 matmuls (ones row x bias row), so phase
   outputs stay in [batch, hidden] layout and evacuate with plain tanh
   in two [64,512] activations per phase (PSUM banks are 2KB/partition).
   Both phases run a SPLIT TAIL: the last lags/k-tiles accumulate into
   the first PSUM bank only, closing it ~1.3us early so its tanh and the
   first PE transposes overlap the second bank's remaining matmuls.
4. Layout flips Z1/Z2 [64b,1024h] -> 8x[128h,64b] use PE transposes
   interleaved with their consumer matmuls (T_k ... k-matmuls), with DVE
   evacuating each transposed tile PSUM->SBUF.
5. Total HBM stream ~2.75MB/core across both HWDGE rings + SWDGE,
   chunked in consumption order so phase 1 starts ~11us in and W2
   overlaps phase 1.  PE warm-up matmuls (throwaway groups in a real
   PSUM bank) ramp the clock during the initial DMA fill.
   Data-parallel over batch: 64 cols/core, no collectives.
"""

import sys

for _p in ("/opt/trn_rl_repo", "/root/.axon_site/_ro/trn_rl_repo"):
    if _p not in sys.path:
        sys.path.append(_p)

import numpy as np
import ml_dtypes

import concourse.bass as bass  # noqa: F401  (bass must import before bacc)
import concourse.mybir as mybir
import concourse.tile as tile
from concourse import bacc
from concourse.bass_utils import run_bass_kernel_spmd

T, BATCH, IN, HID, NCLS = 512, 512, 128, 1024, 10
NCORES = 8
SB = BATCH // NCORES   # batch columns per core
Km = 9                 # exact lags (all fp8-e3m4)
Kc = 22                # corrected lags (tail handled via Gt only)
NT = HID // 128
HH = HID // 2          # 512: psum bank width (fp32)

F32 = mybir.dt.float32
F16 = mybir.dt.float16
F8E3 = mybir.dt.float8e3
F8E4 = mybir.dt.float8e4
NPE3 = ml_dtypes.float8_e3m4
NPE4 = ml_dtypes.float8_e4m3fn
ACT = mybir.ActivationFunctionType

# f16 blob column offsets
XH_O = 0
XH_W = Km * SB                 # 576
W3_O = XH_O + XH_W
W3_W = NT * NCLS               # 80
GT_O = W3_O + W3_W
GT_W = (Kc - Km) * NCLS        # 130
GQ_O = GT_O + GT_W
GQ_W = Km * NCLS               # 90
G1_O = GQ_O + GQ_W
G1_W = NT * NCLS               # 80
ID_O = G1_O + G1_W
F16W = ID_O + 64

# brow (single-partition f16) offsets
B1_O = 0
B2_O = HID
ON_O = 2 * HID
B3R_O = ON_O + SB
BROWW = B3R_O + 16             # 2128

E2 = 32.0                      # W2 power-2 scale (asserted vs host)
ECORR = 256.0                  # correction-column power-2 scale (asserted)

_PROGRAM_CACHE = {}


def _build_program(ncores=NCORES):
    nc = bacc.Bacc(
        "TRN2",
        target_bir_lowering=False,
        debug=False,
        num_devices=ncores,
    )

    F16Bd = nc.dram_tensor("F16B", [128, F16W], F16, kind="ExternalInput").ap()
    BROWd = nc.dram_tensor("BROW", [1, BROWW], F16, kind="ExternalInput").ap()
    D8d = nc.dram_tensor("D8", [128, Km, HID], F8E3, kind="ExternalInput").ap()
    W2d = nc.dram_tensor("W2P", [128, NT, HID], F8E3, kind="ExternalInput").ap()
    XCd = nc.dram_tensor("XC", [128, (Kc - Km) * SB], F8E4, kind="ExternalInput").ap()
    outd = nc.dram_tensor("out", [NCLS, SB], F32, kind="ExternalOutput").ap()

    with tile.TileContext(nc) as tc:
        with (
            tc.tile_pool(name="cst", bufs=1) as cp,
            tc.tile_pool(name="sb", bufs=1) as sp,
            tc.tile_pool(name="psum", bufs=1, space="PSUM") as pp,
        ):
            # ---- SBUF tiles ----
            f16b = cp.tile([128, F16W], F16, tag="f16b")
            brow = cp.tile([1, BROWW], F16, tag="brow")
            d8 = cp.tile([128, Km, HID], F8E3, tag="d8")
            w2 = cp.tile([128, NT, HID], F8E3, tag="w2")
            xc = cp.tile([128, (Kc - Km), SB], F8E4, tag="xc")
            warm = cp.tile([128, HH], F16, tag="warm")
            yt = sp.tile([64, HID], F16, tag="yt")
            yt2 = sp.tile([64, HID], F16, tag="yt2")
            z1t = sp.tile([128, NT, SB], F16, tag="z1t")
            z2t = sp.tile([128, NT, SB], F16, tag="z2t")
            corrall = sp.tile([64, NCLS], F16, tag="corrall")
            ptCsb = sp.tile([NCLS, SB], F16, tag="ptCsb")
            ot = sp.tile([NCLS, SB], F32, tag="ot")

            # ---- DMA issue (order per ring = transfer order).  The
            # scalar HWDGE ring is empirically the fastest; it carries
            # the bulk in consumption order.
            nc.sync.dma_start(f16b[:], F16Bd[:])
            nc.scalar.dma_start(d8[:, 0:3, :], D8d[:, 0:3, :])    # lags 0-2
            nc.scalar.dma_start(d8[:, 3:5, :], D8d[:, 3:5, :])    # lags 3-4
            nc.scalar.dma_start(d8[:, 5:7, :], D8d[:, 5:7, :])    # lags 5-6
            nc.sync.dma_start(d8[:, 7:9, :], D8d[:, 7:9, :])      # lags 7-8
            nc.gpsimd.dma_start(brow[:], BROWd[:])
            nc.gpsimd.dma_start(xc[:], XCd[:])
            nc.scalar.dma_start(w2[:, 0:2, :], W2d[:, 0:2, :])
            nc.scalar.dma_start(w2[:, 2:4, :], W2d[:, 2:4, :])
            nc.scalar.dma_start(w2[:, 4:6, :], W2d[:, 4:6, :])
            nc.scalar.dma_start(w2[:, 6:8, :], W2d[:, 6:8, :])

            # ---- PSUM layout: psA/psB [64,512] (phase 1), psC/psD
            # (phase 2, same 2 slots), psG [64,10] corr cols, psO [10,64],
            # pt pool 2x [128,64] transposes -> 8 banks total.
            psA = pp.tile([64, HH], F32, tag="pA", bufs=1, name="psA")
            psB = pp.tile([64, HH], F32, tag="pB", bufs=1, name="psB")

            # ---- PE warm-up: throwaway groups in psB's bank ----
            nc.vector.memset(warm[:], 0.0)
            for r in range(10):
                n = HH if r < 7 else 128
                nc.tensor.matmul(
                    psB[:, 0:n], warm[:, 0:64], warm[:, 0:n],
                    start=(r == 0), stop=(r == 9),
                )

            # ---- phase 1: Y[64b, 1024h] over Km lags + Gq columns.
            # Lags 0..5 interleave psA/psB (data-paced); the tail lags run
            # psA-first so psA stops ~1.3us early and ACT-A + the first
            # transposes overlap the psB tail matmuls.
            psG = pp.tile([64, NCLS], F32, tag="psG", bufs=1)
            HEAD1 = [0, 1, 2, 3, 4, 5]
            TAILL = [6, 7, 8]
            for gi, g in enumerate(HEAD1):
                xg = f16b[:, XH_O + g * SB : XH_O + (g + 1) * SB]
                nc.tensor.matmul(
                    psA[:], xg, d8[:, g, 0:HH],
                    start=(gi == 0), stop=False,
                )
                nc.tensor.matmul(
                    psB[:], xg, d8[:, g, HH:HID],
                    start=(gi == 0), stop=False,
                )
                nc.tensor.matmul(
                    psG[:], xg, f16b[:, GQ_O + g * NCLS : GQ_O + (g + 1) * NCLS],
                    start=(gi == 0), stop=False,
                )
            ones = brow[0:1, ON_O : ON_O + SB]
            for g in TAILL:
                xg = f16b[:, XH_O + g * SB : XH_O + (g + 1) * SB]
                nc.tensor.matmul(psA[:], xg, d8[:, g, 0:HH], start=False, stop=False)
                nc.tensor.matmul(
                    psG[:], xg, f16b[:, GQ_O + g * NCLS : GQ_O + (g + 1) * NCLS],
                    start=False, stop=False,
                )
            nc.tensor.matmul(
                psA[:], ones, brow[0:1, B1_O : B1_O + HH],
                start=False, stop=True,
            )
            for g in TAILL:
                xg = f16b[:, XH_O + g * SB : XH_O + (g + 1) * SB]
                nc.tensor.matmul(psB[:], xg, d8[:, g, HH:HID], start=False, stop=False)
            nc.tensor.matmul(
                psB[:], ones, brow[0:1, B1_O + HH : B1_O + HID],
                start=False, stop=True,
            )

            # ---- tail-lag corrections into psO [10, 64] ----
            psO = pp.tile([NCLS, SB], F32, tag="psO", bufs=1)
            for i in range(Kc - Km):
                nc.tensor.matmul(
                    psO[:],
                    f16b[:, GT_O + i * NCLS : GT_O + (i + 1) * NCLS],
                    xc[:, i, :],
                    start=(i == 0), stop=False,
                )

            # ---- evacuate phase 1: tanh -> yt (one ACT per bank) ----
            nc.scalar.activation(yt[:, 0:HH], psA[:], ACT.Tanh)
            nc.scalar.activation(yt[:, HH:HID], psB[:], ACT.Tanh)

            # ---- phase 2 with interleaved PE transposes of z1 tiles ----
            psC = pp.tile([64, HH], F32, tag="pA", bufs=1, name="psC")
            psD = pp.tile([64, HH], F32, tag="pB", bufs=1, name="psD")

            def emit_T(zt, src_yt, k, nm):
                pt = pp.tile([128, SB], F16, tag="pt", bufs=2, name=nm)
                nc.tensor.transpose(
                    pt[:], src_yt[:, k * 128 : (k + 1) * 128],
                    f16b[0:64, ID_O : ID_O + 64],
                )
                nc.vector.tensor_copy(zt[:, k, :], pt[:])

            for k in range(4):
                emit_T(z1t, yt, k, f"pt{k}")
            TAIL2 = NT - 2
            for k in range(TAIL2):
                if k + 4 < NT:
                    emit_T(z1t, yt, k + 4, f"pt{k + 4}")
                zk = z1t[:, k, :]
                nc.tensor.matmul(
                    psC[:], zk, w2[:, k, 0:HH],
                    start=(k == 0), stop=False,
                )
                nc.tensor.matmul(
                    psD[:], zk, w2[:, k, HH:HID],
                    start=(k == 0), stop=False,
                )
                nc.tensor.matmul(
                    psG[:], zk, f16b[:, G1_O + k * NCLS : G1_O + (k + 1) * NCLS],
                    start=False, stop=False,
                )
            for k in range(TAIL2, NT):
                zk = z1t[:, k, :]
                nc.tensor.matmul(psC[:], zk, w2[:, k, 0:HH], start=False, stop=False)
                nc.tensor.matmul(
                    psG[:], zk, f16b[:, G1_O + k * NCLS : G1_O + (k + 1) * NCLS],
                    start=False, stop=(k == NT - 1),
                )
            nc.tensor.matmul(
                psC[:], ones, brow[0:1, B2_O : B2_O + HH],
                start=False, stop=True,
            )
            for k in range(TAIL2, NT):
                zk = z1t[:, k, :]
                nc.tensor.matmul(psD[:], zk, w2[:, k, HH:HID], start=False, stop=False)
            nc.tensor.matmul(
                psD[:], ones, brow[0:1, B2_O + HH : B2_O + HID],
                start=False, stop=True,
            )
            

            # ---- evacuate phase 2: tanh(x/e2) -> yt2 ----
            nc.scalar.activation(yt2[:, 0:HH], psC[:], ACT.Tanh, scale=1.0 / E2)
            nc.scalar.activation(yt2[:, HH:HID], psD[:], ACT.Tanh, scale=1.0 / E2)

            # corr columns to fp16 while the out stage runs
            nc.vector.tensor_copy(corrall[:], psG[:])
            # b3 via K=1 matmul (independent of z2 - keep off the tail)
            nc.tensor.matmul(
                psO[:],
                brow[0:1, B3R_O : B3R_O + NCLS],
                ones,
                start=False, stop=False,
            )
            # transpose corr [64,10] -> [10,64] now (psG stopped long ago)
            ptC = pp.tile([128, SB], F16, tag="pt", bufs=2, name="ptC")
            nc.tensor.transpose(
                ptC[0:NCLS, :], corrall[:], f16b[0:64, ID_O : ID_O + 64]
            )
            nc.vector.tensor_copy(ptCsb[:], ptC[0:NCLS, :])

            # ---- out stage: psO += W3 @ z2, transposes interleaved ----
            emit_T(z2t, yt2, 0, "qt0")
            emit_T(z2t, yt2, 1, "qt1")
            for k in range(NT):
                if k + 2 < NT:
                    emit_T(z2t, yt2, k + 2, f"qt{k + 2}")
                nc.tensor.matmul(
                    psO[:],
                    f16b[:, W3_O + k * NCLS : W3_O + (k + 1) * NCLS],
                    z2t[:, k, :],
                    start=False, stop=(k == NT - 1),
                )
            # ot = ptCsb * (1/e_corr) + psO
            nc.vector.scalar_tensor_tensor(
                ot[:], ptCsb[:], 1.0 / ECORR, psO[:],
                mybir.AluOpType.mult, mybir.AluOpType.add,
            )
            nc.sync.dma_start(outd[:], ot[:])

    nc.compile()
    return nc


def _prep_weights(A, B, bias, W1, b1, W2, b2, W3, b3):
    """Host fp64 weight-only precompute (c1/c2 calibrated on synthetic
    gaussian x matching the spec'd input distribution, never the real x)."""
    B64 = B.astype(np.float64)
    W164 = W1.astype(np.float64)
    A64 = A.astype(np.float64)
    b64 = bias.astype(np.float64)
    W264 = W2.astype(np.float64)
    W364 = W3.astype(np.float64)

    Ds, M = [], A64.copy()
    for g in range(Kc):
        Ds.append(W164 @ M)
        M = B64 @ M
    Dsum = W164 @ np.linalg.solve(np.eye(HID) - B64, A64)
    b1f = b1.astype(np.float64) - Dsum @ b64

    rng = np.random.default_rng(12345)
    xcal = rng.standard_normal((Kc, 256, IN))
    Ycal = sum(xcal[g] @ Ds[g].T for g in range(Kc))
    c1 = float((1 - np.tanh(Ycal + b1f) ** 2).mean())
    y2cal = np.tanh(Ycal + b1f) @ W264.T + b2.astype(np.float64)
    c2 = float((1 - np.tanh(y2cal) ** 2).mean())

    D8 = np.empty((IN, Km, HID), NPE3)
    lagE, e_lag = [], []
    for g in range(Km):
        m = np.abs(Ds[g]).max()
        e = 2.0 ** np.clip(np.floor(np.log2(8.0 / m)), 0, 6)
        Dq = (Ds[g].T * e).astype(NPE3)
        D8[:, g, :] = Dq
        e_lag.append(e)
        lagE.append(e * Ds[g].T - Dq.astype(np.float64))

    mW2 = np.abs(W264).max()
    e2 = 2.0 ** np.floor(np.log2(8.0 / mW2))
    W2q = (W264.T * e2).astype(NPE3)              # [k, m]
    E2m = W264.T - W2q.astype(np.float64) / e2
    W2P = np.empty((IN, NT, HID), NPE3)
    for k in range(NT):
        W2P[:, k, :] = W2q[k * 128 : (k + 1) * 128, :]

    CWm = c1 * c2 * (W264.T @ W364.T)
    Gq = [lagE[g] @ CWm for g in range(Km)]       # [IN, 10] at xq scale
    G1 = c2 * (E2m @ W364.T)                      # [k, 10] applied to z1
    Gt = [Ds[g].T @ CWm for g in range(Km, Kc)]   # [IN, 10] at true x scale

    gmax = max(max(np.abs(g_).max() for g_ in Gq), np.abs(G1).max())
    e_corr = 2.0 ** np.floor(np.log2(8.0 / gmax))

    brow = np.zeros((1, BROWW), np.float16)
    brow[0, B1_O : B1_O + HID] = b1f.astype(np.float16)
    brow[0, B2_O : B2_O + HID] = (b2.astype(np.float64) * e2).astype(np.float16)
    brow[0, ON_O : ON_O + SB] = 1.0
    brow[0, B3R_O : B3R_O + NCLS] = b3.astype(np.float16)

    f16c = np.zeros((128, F16W), np.float16)
    W3T = W364.T.astype(np.float16)               # [HID, 10]
    for k in range(NT):
        f16c[:, W3_O + k * NCLS : W3_O + (k + 1) * NCLS] = (
            W3T[k * 128 : (k + 1) * 128, :]
        )
    for i in range(Kc - Km):
        f16c[:, GT_O + i * NCLS : GT_O + (i + 1) * NCLS] = Gt[i].astype(np.float16)
    for g in range(Km):
        f16c[:, GQ_O + g * NCLS : GQ_O + (g + 1) * NCLS] = (
            (Gq[g] * e_corr).astype(np.float16)
        )
    for k in range(NT):
        f16c[:, G1_O + k * NCLS : G1_O + (k + 1) * NCLS] = (
            (G1[k * 128 : (k + 1) * 128, :] * e_corr).astype(np.float16)
        )
    f16c[0:64, ID_O : ID_O + 64] = np.eye(64, dtype=np.float16)

    return {
        "e_lag": e_lag, "e2": e2, "e_corr": e_corr,
        "D8": D8, "W2P": W2P, "brow": brow, "f16c": f16c,
        "c1": c1, "c2": c2,
    }


def _prep_inputs(x, wp, ncores=NCORES):
    in_maps = []
    for c in range(ncores):
        bsl = slice(c * SB, (c + 1) * SB)
        f16b = wp["f16c"].copy()
        for g in range(Km):
            f16b[:, XH_O + g * SB : XH_O + (g + 1) * SB] = (
                x[T - 1 - g, bsl, :].T / wp["e_lag"][g]
            ).astype(np.float16)
        XC = np.empty((IN, (Kc - Km) * SB), NPE4)
        for i, g in enumerate(range(Km, Kc)):
            XC[:, i * SB : (i + 1) * SB] = x[T - 1 - g, bsl, :].T.astype(NPE4)
        in_maps.append(
            {
                "F16B": f16b,
                "BROW": wp["brow"],
                "D8": wp["D8"],
                "W2P": wp["W2P"],
                "XC": XC,
            }
        )
    return in_maps


def kernel(x, A, B, bias, W1, b1, W2, b2, W3, b3, _trace=False):
    wp = _prep_weights(A, B, bias, W1, b1, W2, b2, W3, b3)
    assert wp["e2"] == E2, "activation scale 1/e2 hardcoded in program"
    assert wp["e_corr"] == ECORR, "1/e_corr hardcoded in program"
    if "nc" not in _PROGRAM_CACHE:
        _PROGRAM_CACHE["nc"] = _build_program()
    nc = _PROGRAM_CACHE["nc"]
    in_maps = _prep_inputs(x, wp)
    res = run_bass_kernel_spmd(nc, in_maps, list(range(NCORES)), trace=_trace)
    _PROGRAM_CACHE["last_result"] = res
    out = np.empty((BATCH, NCLS), np.float32)
    for c in range(NCORES):
        out[c * SB : (c + 1) * SB, :] = res.results[c]["out"].T
    return out


# revision 43
# speedup vs baseline: 1.1401x; 1.0687x over previous
"""Trainium2 Bass kernel for LAES linear recurrence + deep readout (v4).

Math: h_t = (x_t - bias) @ A.T + h_{t-1} @ B.T  (T=512 steps, h0=0),
then out = tanh(tanh(h@W1.T+b1)@W2.T+b2)@W3.T+b3.

Design (v1 37.5us -> v2 36.5 -> v3 35.2 -> v4 32.5 -> v6 ~31.1us typ):
1. Whole pre-tanh pipeline is linear in x: Y = sum_g D_g @ (x_{T-1-g}-bias),
   D_g = W1 B^g A (host fp64 weight precompute).  Main lags g < Km=10
   stream in fp8-e3m4 (scale target 8 -> 1.3% per-entry rms error).
2. Linearized corrections (weights-only; c1,c2 calibrated on synthetic
   gaussian x): every approximation error E (lag quant, W2 quant,
   truncated tail lags 10..21) maps to output space as a [*,10] matrix
   G ~ c1*c2*E.T@W2.T@W3.T, accumulated on-device by tiny matmuls that
   reuse already-loaded PE weights.  This lets W2 stream in fp8-e3m4
   (1MB instead of 2MB) and truncates the recurrence at 10 exact lags.
3. Biases enter PSUM via K=1 matmuls (ones row x bias row), so phase
   outputs stay in [batch, hidden] layout and evacuate with plain tanh
   in two [64,512] activations per phase (PSUM banks are 2KB/partition).
   Both phases run a SPLIT TAIL: the last lags/k-tiles accumulate into
   the first PSUM bank only, closing it ~1.3us early so its tanh and the
   first PE transposes overlap the second bank's remaining matmuls.
4. Layout flips Z1/Z2 [64b,1024h] -> 8x[128h,64b] use PE transposes
   interleaved with their consumer matmuls (T_k ... k-matmuls), with DVE
   evacuating each transposed tile PSUM->SBUF.
5. Total HBM stream ~2.75MB/core across both HWDGE rings + SWDGE,
   chunked in consumption order so phase 1 starts ~11us in and W2
   overlaps phase 1.  PE warm-up matmuls (throwaway groups in a real
   PSUM bank) ramp the clock during the initial DMA fill.
   Data-parallel over batch: 64 cols/core, no collectives.
"""

import sys

for _p in ("/opt/trn_rl_repo", "/root/.axon_site/_ro/trn_rl_repo"):
    if _p not in sys.path:
        sys.path.append(_p)

import numpy as np
import ml_dtypes

import concourse.bass as bass  # noqa: F401  (bass must import before bacc)
import concourse.mybir as mybir
import concourse.tile as tile
from concourse import bacc
from concourse.bass_utils import run_bass_kernel_spmd

T, BATCH, IN, HID, NCLS = 512, 512, 128, 1024, 10
NCORES = 8
SB = BATCH // NCORES   # batch columns per core
Km = 9                 # exact lags (all fp8-e3m4)
Kc = 22                # corrected lags (tail handled via Gt only)
NT = HID // 128
HH = HID // 2          # 512: psum bank width (fp32)

F32 = mybir.dt.float32
F16 = mybir.dt.float16
F8E3 = mybir.dt.float8e3
F8E4 = mybir.dt.float8e4
NPE3 = ml_dtypes.float8_e3m4
NPE4 = ml_dtypes.float8_e4m3fn
ACT = mybir.ActivationFunctionType

# f16 blob column offsets
XH_O = 0
XH_W = Km * SB                 # 576
W3_O = XH_O + XH_W
W3_W = NT * NCLS               # 80
GT_O = W3_O + W3_W
GT_W = (Kc - Km) * NCLS        # 130
GQ_O = GT_O + GT_W
GQ_W = Km * NCLS               # 90
G1_O = GQ_O + GQ_W
G1_W = NT * NCLS               # 80
ID_O = G1_O + G1_W
F16W = ID_O + 64

# brow (single-partition f16) offsets
B1_O = 0
B2_O = HID
ON_O = 2 * HID
B3R_O = ON_O + SB
BROWW = B3R_O + 16             # 2128

E2 = 32.0                      # W2 power-2 scale (asserted vs host)
ECORR = 256.0                  # correction-column power-2 scale (asserted)

_PROGRAM_CACHE = {}


def _build_program(ncores=NCORES):
    nc = bacc.Bacc(
        "TRN2",
        target_bir_lowering=False,
        debug=False,
        num_devices=ncores,
    )

    F16Bd = nc.dram_tensor("F16B", [128, F16W], F16, kind="ExternalInput").ap()
    BROWd = nc.dram_tensor("BROW", [1, BROWW], F16, kind="ExternalInput").ap()
    D8d = nc.dram_tensor("D8", [128, Km, HID], F8E3, kind="ExternalInput").ap()
    W2d = nc.dram_tensor("W2P", [128, NT, HID], F8E3, kind="ExternalInput").ap()
    XCd = nc.dram_tensor("XC", [128, (Kc - Km) * SB], F8E4, kind="ExternalInput").ap()
    outd = nc.dram_tensor("out", [NCLS, SB], F32, kind="ExternalOutput").ap()

    with tile.TileContext(nc) as tc:
        with (
            tc.tile_pool(name="cst", bufs=1) as cp,
            tc.tile_pool(name="sb", bufs=1) as sp,
            tc.tile_pool(name="psum", bufs=1, space="PSUM") as pp,
        ):
            # ---- SBUF tiles ----
            f16b = cp.tile([128, F16W], F16, tag="f16b")
            brow = cp.tile([1, BROWW], F16, tag="brow")
            d8 = cp.tile([128, Km, HID], F8E3, tag="d8")
            w2 = cp.tile([128, NT, HID], F8E3, tag="w2")
            xc = cp.tile([128, (Kc - Km), SB], F8E4, tag="xc")
            warm = cp.tile([128, HH], F16, tag="warm")
            yt = sp.tile([64, HID], F16, tag="yt")
            yt2 = sp.tile([64, HID], F16, tag="yt2")
            z1t = sp.tile([128, NT, SB], F16, tag="z1t")
            z2t = sp.tile([128, NT, SB], F16, tag="z2t")
            corrall = sp.tile([64, NCLS], F16, tag="corrall")
            ptCsb = sp.tile([NCLS, SB], F16, tag="ptCsb")
            ot = sp.tile([NCLS, SB], F32, tag="ot")

            # ---- DMA issue (order per ring = transfer order).  The
            # scalar HWDGE ring is empirically the fastest; it carries
            # the bulk in consumption order.
            nc.sync.dma_start(f16b[:], F16Bd[:])
            nc.scalar.dma_start(d8[:, 0:3, :], D8d[:, 0:3, :])    # lags 0-2
            nc.scalar.dma_start(d8[:, 3:5, :], D8d[:, 3:5, :])    # lags 3-4
            nc.scalar.dma_start(d8[:, 5:7, :], D8d[:, 5:7, :])    # lags 5-6
            nc.sync.dma_start(d8[:, 7:9, :], D8d[:, 7:9, :])      # lags 7-8
            nc.gpsimd.dma_start(brow[:], BROWd[:])
            nc.gpsimd.dma_start(xc[:], XCd[:])
            nc.scalar.dma_start(w2[:, 0:2, :], W2d[:, 0:2, :])
            nc.scalar.dma_start(w2[:, 2:4, :], W2d[:, 2:4, :])
            nc.scalar.dma_start(w2[:, 4:6, :], W2d[:, 4:6, :])
            nc.scalar.dma_start(w2[:, 6:8, :], W2d[:, 6:8, :])

            # ---- PSUM layout: psA/psB [64,512] (phase 1), psC/psD
            # (phase 2, same 2 slots), psG [64,10] corr cols, psO [10,64],
            # pt pool 2x [128,64] transposes -> 8 banks total.
            psA = pp.tile([64, HH], F32, tag="pA", bufs=1, name="psA")
            psB = pp.tile([64, HH], F32, tag="pB", bufs=1, name="psB")

            # ---- PE warm-up: throwaway groups in psB's bank ----
            nc.vector.memset(warm[:], 0.0)
            for r in range(10):
                n = HH if r < 7 else 128
                nc.tensor.matmul(
                    psB[:, 0:n], warm[:, 0:64], warm[:, 0:n],
                    start=(r == 0), stop=(r == 9),
                )

            # ---- phase 1: Y[64b, 1024h] over Km lags + Gq columns.
            # Lags 0..5 interleave psA/psB (data-paced); the tail lags run
            # psA-first so psA stops ~1.3us early and ACT-A + the first
            # transposes overlap the psB tail matmuls.
            psG = pp.tile([64, NCLS], F32, tag="psG", bufs=1)
            HEAD1 = [0, 1, 2, 3, 4, 5]
            TAILL = [6, 7, 8]
            for gi, g in enumerate(HEAD1):
                xg = f16b[:, XH_O + g * SB : XH_O + (g + 1) * SB]
                nc.tensor.matmul(
                    psA[:], xg, d8[:, g, 0:HH],
                    start=(gi == 0), stop=False,
                )
                nc.tensor.matmul(
                    psB[:], xg, d8[:, g, HH:HID],
                    start=(gi == 0), stop=False,
                )
                nc.tensor.matmul(
                    psG[:], xg, f16b[:, GQ_O + g * NCLS : GQ_O + (g + 1) * NCLS],
                    start=(gi == 0), stop=False,
                )
            ones = brow[0:1, ON_O : ON_O + SB]
            for g in TAILL:
                xg = f16b[:, XH_O + g * SB : XH_O + (g + 1) * SB]
                nc.tensor.matmul(psA[:], xg, d8[:, g, 0:HH], start=False, stop=False)
                nc.tensor.matmul(
                    psG[:], xg, f16b[:, GQ_O + g * NCLS : GQ_O + (g + 1) * NCLS],
                    start=False, stop=False,
                )
            nc.tensor.matmul(
                psA[:], ones, brow[0:1, B1_O : B1_O + HH],
                start=False, stop=True,
            )
            for g in TAILL:
                xg = f16b[:, XH_O + g * SB : XH_O + (g + 1) * SB]
                nc.tensor.matmul(psB[:], xg, d8[:, g, HH:HID], start=False, stop=False)
            nc.tensor.matmul(
                psB[:], ones, brow[0:1, B1_O + HH : B1_O + HID],
                start=False, stop=True,
            )

            # ---- tail-lag corrections into psO [10, 64] ----
            psO = pp.tile([NCLS, SB], F32, tag="psO", bufs=1)
            for i in range(Kc - Km):
                nc.tensor.matmul(
                    psO[:],
                    f16b[:, GT_O + i * NCLS : GT_O + (i + 1) * NCLS],
                    xc[:, i, :],
                    start=(i == 0), stop=False,
                )

            # ---- evacuate phase 1: tanh -> yt (one ACT per bank) ----
            nc.scalar.activation(yt[:, 0:HH], psA[:], ACT.Tanh)
            nc.scalar.activation(yt[:, HH:HID], psB[:], ACT.Tanh)

            # ---- phase 2 with interleaved PE transposes of z1 tiles ----
            psC = pp.tile([64, HH], F32, tag="pA", bufs=1, name="psC")
            psD = pp.tile([64, HH], F32, tag="pB", bufs=1, name="psD")

            def emit_T(zt, src_yt, k, nm):
                pt = pp.tile([128, SB], F16, tag="pt", bufs=3, name=nm)
                nc.tensor.transpose(
                    pt[:], src_yt[:, k * 128 : (k + 1) * 128],
                    f16b[0:64, ID_O : ID_O + 64],
                )
                nc.vector.tensor_copy(zt[:, k, :], pt[:])

            for k in range(4):
                emit_T(z1t, yt, k, f"pt{k}")
            TAIL2 = NT - 2
            for k in range(TAIL2):
                if k + 4 < NT:
                    emit_T(z1t, yt, k + 4, f"pt{k + 4}")
                zk = z1t[:, k, :]
                nc.tensor.matmul(
                    psC[:], zk, w2[:, k, 0:HH],
                    start=(k == 0), stop=False,
                )
                nc.tensor.matmul(
                    psD[:], zk, w2[:, k, HH:HID],
                    start=(k == 0), stop=False,
                )
                nc.tensor.matmul(
                    psG[:], zk, f16b[:, G1_O + k * NCLS : G1_O + (k + 1) * NCLS],
                    start=False, stop=False,
                )
            for k in range(TAIL2, NT):
                zk = z1t[:, k, :]
                nc.tensor.matmul(psC[:], zk, w2[:, k, 0:HH], start=False, stop=False)
                nc.tensor.matmul(
                    psG[:], zk, f16b[:, G1_O + k * NCLS : G1_O + (k + 1) * NCLS],
                    start=False, stop=(k == NT - 1),
                )
            nc.tensor.matmul(
                psC[:], ones, brow[0:1, B2_O : B2_O + HH],
                start=False, stop=True,
            )
            for k in range(TAIL2, NT):
                zk = z1t[:, k, :]
                nc.tensor.matmul(psD[:], zk, w2[:, k, HH:HID], start=False, stop=False)
            nc.tensor.matmul(
                psD[:], ones, brow[0:1, B2_O + HH : B2_O + HID],
                start=False, stop=True,
            )
            

            # ---- evacuate phase 2: tanh(x/e2) -> yt2 ----
            nc.scalar.activation(yt2[:, 0:HH], psC[:], ACT.Tanh, scale=1.0 / E2)
            nc.scalar.activation(yt2[:, HH:HID], psD[:], ACT.Tanh, scale=1.0 / E2)

            # corr columns to fp16 while the out stage runs
            nc.vector.tensor_copy(corrall[:], psG[:])
            # b3 via K=1 matmul (independent of z2 - keep off the tail)
            nc.tensor.matmul(
                psO[:],
                brow[0:1, B3R_O : B3R_O + NCLS],
                ones,
                start=False, stop=False,
            )
            # transpose corr [64,10] -> [10,64] now (psG stopped long ago)
            ptC = pp.tile([128, SB], F16, tag="pt", bufs=3, name="ptC")
            nc.tensor.transpose(
                ptC[0:NCLS, :], corrall[:], f16b[0:64, ID_O : ID_O + 64]
            )
            nc.vector.tensor_copy(ptCsb[:], ptC[0:NCLS, :])

            # ---- out stage: psO += W3 @ z2, transposes interleaved ----
            emit_T(z2t, yt2, 0, "qt0")
            emit_T(z2t, yt2, 1, "qt1")
            for k in range(NT):
                if k + 2 < NT:
                    emit_T(z2t, yt2, k + 2, f"qt{k + 2}")
                nc.tensor.matmul(
                    psO[:],
                    f16b[:, W3_O + k * NCLS : W3_O + (k + 1) * NCLS],
                    z2t[:, k, :],
                    start=False, stop=(k == NT - 1),
                )
            # ot = ptCsb * (1/e_corr) + psO
            nc.vector.scalar_tensor_tensor(
                ot[:], ptCsb[:], 1.0 / ECORR, psO[:],
                mybir.AluOpType.mult, mybir.AluOpType.add,
            )
            nc.sync.dma_start(outd[:], ot[:])

    nc.compile()
    return nc


def _prep_weights(A, B, bias, W1, b1, W2, b2, W3, b3):
    """Host fp64 weight-only precompute (c1/c2 calibrated on synthetic
    gaussian x matching the spec'd input distribution, never the real x)."""
    B64 = B.astype(np.float64)
    W164 = W1.astype(np.float64)
    A64 = A.astype(np.float64)
    b64 = bias.astype(np.float64)
    W264 = W2.astype(np.float64)
    W364 = W3.astype(np.float64)

    Ds, M = [], A64.copy()
    for g in range(Kc):
        Ds.append(W164 @ M)
        M = B64 @ M
    Dsum = W164 @ np.linalg.solve(np.eye(HID) - B64, A64)
    b1f = b1.astype(np.float64) - Dsum @ b64

    rng = np.random.default_rng(12345)
    xcal = rng.standard_normal((Kc, 256, IN))
    Ycal = sum(xcal[g] @ Ds[g].T for g in range(Kc))
    c1 = float((1 - np.tanh(Ycal + b1f) ** 2).mean())
    y2cal = np.tanh(Ycal + b1f) @ W264.T + b2.astype(np.float64)
    c2 = float((1 - np.tanh(y2cal) ** 2).mean())

    D8 = np.empty((IN, Km, HID), NPE3)
    lagE, e_lag = [], []
    for g in range(Km):
        m = np.abs(Ds[g]).max()
        e = 2.0 ** np.clip(np.floor(np.log2(8.0 / m)), 0, 6)
        Dq = (Ds[g].T * e).astype(NPE3)
        D8[:, g, :] = Dq
        e_lag.append(e)
        lagE.append(e * Ds[g].T - Dq.astype(np.float64))

    mW2 = np.abs(W264).max()
    e2 = 2.0 ** np.floor(np.log2(8.0 / mW2))
    W2q = (W264.T * e2).astype(NPE3)              # [k, m]
    E2m = W264.T - W2q.astype(np.float64) / e2
    W2P = np.empty((IN, NT, HID), NPE3)
    for k in range(NT):
        W2P[:, k, :] = W2q[k * 128 : (k + 1) * 128, :]

    CWm = c1 * c2 * (W264.T @ W364.T)
    Gq = [lagE[g] @ CWm for g in range(Km)]       # [IN, 10] at xq scale
    G1 = c2 * (E2m @ W364.T)                      # [k, 10] applied to z1
    Gt = [Ds[g].T @ CWm for g in range(Km, Kc)]   # [IN, 10] at true x scale

    gmax = max(max(np.abs(g_).max() for g_ in Gq), np.abs(G1).max())
    e_corr = 2.0 ** np.floor(np.log2(8.0 / gmax))

    brow = np.zeros((1, BROWW), np.float16)
    brow[0, B1_O : B1_O + HID] = b1f.astype(np.float16)
    brow[0, B2_O : B2_O + HID] = (b2.astype(np.float64) * e2).astype(np.float16)
    brow[0, ON_O : ON_O + SB] = 1.0
    brow[0, B3R_O : B3R_O + NCLS] = b3.astype(np.float16)

    f16c = np.zeros((128, F16W), np.float16)
    W3T = W364.T.astype(np.float16)               # [HID, 10]
    for k in range(NT):
        f16c[:, W3_O + k * NCLS : W3_O + (k + 1) * NCLS] = (
            W3T[k * 128 : (k + 1) * 128, :]
        )
    for i in range(Kc - Km):
        f16c[:, GT_O + i * NCLS : GT_O + (i + 1) * NCLS] = Gt[i].astype(np.float16)
    for g in range(Km):
        f16c[:, GQ_O + g * NCLS : GQ_O + (g + 1) * NCLS] = (
            (Gq[g] * e_corr).astype(np.float16)
        )
    for k in range(NT):
        f16c[:, G1_O + k * NCLS : G1_O + (k + 1) * NCLS] = (
            (G1[k * 128 : (k + 1) * 128, :] * e_corr).astype(np.float16)
        )
    f16c[0:64, ID_O : ID_O + 64] = np.eye(64, dtype=np.float16)

    return {
        "e_lag": e_lag, "e2": e2, "e_corr": e_corr,
        "D8": D8, "W2P": W2P, "brow": brow, "f16c": f16c,
        "c1": c1, "c2": c2,
    }


def _prep_inputs(x, wp, ncores=NCORES):
    in_maps = []
    for c in range(ncores):
        bsl = slice(c * SB, (c + 1) * SB)
        f16b = wp["f16c"].copy()
        for g in range(Km):
            f16b[:, XH_O + g * SB : XH_O + (g + 1) * SB] = (
                x[T - 1 - g, bsl, :].T / wp["e_lag"][g]
            ).astype(np.float16)
        XC = np.empty((IN, (Kc - Km) * SB), NPE4)
        for i, g in enumerate(range(Km, Kc)):
            XC[:, i * SB : (i + 1) * SB] = x[T - 1 - g, bsl, :].T.astype(NPE4)
        in_maps.append(
            {
                "F16B": f16b,
                "BROW": wp["brow"],
                "D8": wp["D8"],
                "W2P": wp["W2P"],
                "XC": XC,
            }
        )
    return in_maps


def kernel(x, A, B, bias, W1, b1, W2, b2, W3, b3, _trace=False):
    wp = _prep_weights(A, B, bias, W1, b1, W2, b2, W3, b3)
    assert wp["e2"] == E2, "activation scale 1/e2 hardcoded in program"
    assert wp["e_corr"] == ECORR, "1/e_corr hardcoded in program"
    if "nc" not in _PROGRAM_CACHE:
        _PROGRAM_CACHE["nc"] = _build_program()
    nc = _PROGRAM_CACHE["nc"]
    in_maps = _prep_inputs(x, wp)
    res = run_bass_kernel_spmd(nc, in_maps, list(range(NCORES)), trace=_trace)
    _PROGRAM_CACHE["last_result"] = res
    out = np.empty((BATCH, NCLS), np.float32)
    for c in range(NCORES):
        out[c * SB : (c + 1) * SB, :] = res.results[c]["out"].T
    return out


# revision 44
# speedup vs baseline: 1.1515x; 1.0100x over previous
"""Trainium2 Bass kernel for LAES linear recurrence + deep readout (v4).

Math: h_t = (x_t - bias) @ A.T + h_{t-1} @ B.T  (T=512 steps, h0=0),
then out = tanh(tanh(h@W1.T+b1)@W2.T+b2)@W3.T+b3.

Design (v1 37.5us -> v2 36.5 -> v3 35.2 -> v4 32.5 -> v6 ~31.1us typ):
1. Whole pre-tanh pipeline is linear in x: Y = sum_g D_g @ (x_{T-1-g}-bias),
   D_g = W1 B^g A (host fp64 weight precompute).  Main lags g < Km=10
   stream in fp8-e3m4 (scale target 8 -> 1.3% per-entry rms error).
2. Linearized corrections (weights-only; c1,c2 calibrated on synthetic
   gaussian x): every approximation error E (lag quant, W2 quant,
   truncated tail lags 10..21) maps to output space as a [*,10] matrix
   G ~ c1*c2*E.T@W2.T@W3.T, accumulated on-device by tiny matmuls that
   reuse already-loaded PE weights.  This lets W2 stream in fp8-e3m4
   (1MB instead of 2MB) and truncates the recurrence at 10 exact lags.
3. Biases enter PSUM via K=1 matmuls (ones row x bias row), so phase
   outputs stay in [batch, hidden] layout and evacuate with plain tanh
   in two [64,512] activations per phase (PSUM banks are 2KB/partition).
   Both phases run a SPLIT TAIL: the last lags/k-tiles accumulate into
   the first PSUM bank only, closing it ~1.3us early so its tanh and the
   first PE transposes overlap the second bank's remaining matmuls.
4. Layout flips Z1/Z2 [64b,1024h] -> 8x[128h,64b] use PE transposes
   interleaved with their consumer matmuls (T_k ... k-matmuls), with DVE
   evacuating each transposed tile PSUM->SBUF.
5. Total HBM stream ~2.75MB/core across both HWDGE rings + SWDGE,
   chunked in consumption order so phase 1 starts ~11us in and W2
   overlaps phase 1.  PE warm-up matmuls (throwaway groups in a real
   PSUM bank) ramp the clock during the initial DMA fill.
   Data-parallel over batch: 64 cols/core, no collectives.
"""

import sys

for _p in ("/opt/trn_rl_repo", "/root/.axon_site/_ro/trn_rl_repo"):
    if _p not in sys.path:
        sys.path.append(_p)

import numpy as np
import ml_dtypes

import concourse.bass as bass  # noqa: F401  (bass must import before bacc)
import concourse.mybir as mybir
import concourse.tile as tile
from concourse import bacc
from concourse.bass_utils import run_bass_kernel_spmd

T, BATCH, IN, HID, NCLS = 512, 512, 128, 1024, 10
NCORES = 8
SB = BATCH // NCORES   # batch columns per core
Km = 9                 # exact lags (all fp8-e3m4)
Kc = 22                # corrected lags (tail handled via Gt only)
NT = HID // 128
HH = HID // 2          # 512: psum bank width (fp32)

F32 = mybir.dt.float32
F16 = mybir.dt.float16
F8E3 = mybir.dt.float8e3
F8E4 = mybir.dt.float8e4
NPE3 = ml_dtypes.float8_e3m4
NPE4 = ml_dtypes.float8_e4m3fn
ACT = mybir.ActivationFunctionType

# f16 blob column offsets
XH_O = 0
XH_W = Km * SB                 # 576
W3_O = XH_O + XH_W
W3_W = NT * NCLS               # 80
GT_O = W3_O + W3_W
GT_W = (Kc - Km) * NCLS        # 130
GQ_O = GT_O + GT_W
GQ_W = Km * NCLS               # 90
G1_O = GQ_O + GQ_W
G1_W = NT * NCLS               # 80
ID_O = G1_O + G1_W
F16W = ID_O + 64

# brow (single-partition f16) offsets
B1_O = 0
B2_O = HID
ON_O = 2 * HID
B3R_O = ON_O + SB
BROWW = B3R_O + 16             # 2128

E2 = 32.0                      # W2 power-2 scale (asserted vs host)
ECORR = 256.0                  # correction-column power-2 scale (asserted)

_PROGRAM_CACHE = {}


def _build_program(ncores=NCORES):
    nc = bacc.Bacc(
        "TRN2",
        target_bir_lowering=False,
        debug=False,
        num_devices=ncores,
    )

    F16Bd = nc.dram_tensor("F16B", [128, F16W], F16, kind="ExternalInput").ap()
    BROWd = nc.dram_tensor("BROW", [1, BROWW], F16, kind="ExternalInput").ap()
    D8d = nc.dram_tensor("D8", [128, Km, HID], F8E3, kind="ExternalInput").ap()
    W2d = nc.dram_tensor("W2P", [128, NT, HID], F8E3, kind="ExternalInput").ap()
    XCd = nc.dram_tensor("XC", [128, (Kc - Km) * SB], F8E4, kind="ExternalInput").ap()
    outd = nc.dram_tensor("out", [NCLS, SB], F32, kind="ExternalOutput").ap()

    with tile.TileContext(nc) as tc:
        with (
            tc.tile_pool(name="cst", bufs=1) as cp,
            tc.tile_pool(name="sb", bufs=1) as sp,
            tc.tile_pool(name="psum", bufs=1, space="PSUM") as pp,
        ):
            # ---- SBUF tiles ----
            f16b = cp.tile([128, F16W], F16, tag="f16b")
            brow = cp.tile([1, BROWW], F16, tag="brow")
            d8 = cp.tile([128, Km, HID], F8E3, tag="d8")
            w2 = cp.tile([128, NT, HID], F8E3, tag="w2")
            xc = cp.tile([128, (Kc - Km), SB], F8E4, tag="xc")
            warm = cp.tile([128, HH], F16, tag="warm")
            yt = sp.tile([64, HID], F16, tag="yt")
            yt2 = sp.tile([64, HID], F16, tag="yt2")
            z1t = sp.tile([128, NT, SB], F16, tag="z1t")
            z2t = sp.tile([128, NT, SB], F16, tag="z2t")
            corrall = sp.tile([64, NCLS], F16, tag="corrall")
            ptCsb = sp.tile([NCLS, SB], F16, tag="ptCsb")
            ot = sp.tile([NCLS, SB], F32, tag="ot")

            # ---- DMA issue (order per ring = transfer order).  The
            # scalar HWDGE ring is empirically the fastest; it carries
            # the bulk in consumption order.
            nc.sync.dma_start(f16b[:], F16Bd[:])
            nc.scalar.dma_start(d8[:, 0:3, :], D8d[:, 0:3, :])    # lags 0-2
            nc.scalar.dma_start(d8[:, 3:5, :], D8d[:, 3:5, :])    # lags 3-4
            nc.scalar.dma_start(d8[:, 5:7, :], D8d[:, 5:7, :])    # lags 5-6
            nc.sync.dma_start(d8[:, 7:9, :], D8d[:, 7:9, :])      # lags 7-8
            nc.gpsimd.dma_start(brow[:], BROWd[:])
            nc.gpsimd.dma_start(xc[:], XCd[:])
            nc.scalar.dma_start(w2[:, 0:2, :], W2d[:, 0:2, :])
            nc.scalar.dma_start(w2[:, 2:4, :], W2d[:, 2:4, :])
            nc.scalar.dma_start(w2[:, 4:6, :], W2d[:, 4:6, :])
            nc.scalar.dma_start(w2[:, 6:8, :], W2d[:, 6:8, :])

            # ---- PSUM layout: psA/psB [64,512] (phase 1), psC/psD
            # (phase 2, same 2 slots), psG [64,10] corr cols, psO [10,64],
            # pt pool 2x [128,64] transposes -> 8 banks total.
            psA = pp.tile([64, HH], F32, tag="pA", bufs=1, name="psA")
            psB = pp.tile([64, HH], F32, tag="pB", bufs=1, name="psB")

            # ---- PE warm-up: throwaway groups in psB's bank ----
            nc.vector.memset(warm[:], 0.0)
            for r in range(10):
                n = HH if r < 7 else 128
                nc.tensor.matmul(
                    psB[:, 0:n], warm[:, 0:64], warm[:, 0:n],
                    start=(r == 0), stop=(r == 9),
                )

            # ---- phase 1: Y[64b, 1024h] over Km lags + Gq columns.
            # Lags 0..5 interleave psA/psB (data-paced); the tail lags run
            # psA-first so psA stops ~1.3us early and ACT-A + the first
            # transposes overlap the psB tail matmuls.
            psG = pp.tile([64, NCLS], F32, tag="psG", bufs=1)
            HEAD1 = [0, 1, 2, 3, 4, 5]
            TAILL = [6, 7, 8]
            for gi, g in enumerate(HEAD1):
                xg = f16b[:, XH_O + g * SB : XH_O + (g + 1) * SB]
                nc.tensor.matmul(
                    psA[:], xg, d8[:, g, 0:HH],
                    start=(gi == 0), stop=False,
                )
                nc.tensor.matmul(
                    psB[:], xg, d8[:, g, HH:HID],
                    start=(gi == 0), stop=False,
                )
                nc.tensor.matmul(
                    psG[:], xg, f16b[:, GQ_O + g * NCLS : GQ_O + (g + 1) * NCLS],
                    start=(gi == 0), stop=False,
                )
            ones = brow[0:1, ON_O : ON_O + SB]
            for g in TAILL:
                xg = f16b[:, XH_O + g * SB : XH_O + (g + 1) * SB]
                nc.tensor.matmul(psA[:], xg, d8[:, g, 0:HH], start=False, stop=False)
                nc.tensor.matmul(
                    psG[:], xg, f16b[:, GQ_O + g * NCLS : GQ_O + (g + 1) * NCLS],
                    start=False, stop=False,
                )
            nc.tensor.matmul(
                psA[:], ones, brow[0:1, B1_O : B1_O + HH],
                start=False, stop=True,
            )
            for g in TAILL:
                xg = f16b[:, XH_O + g * SB : XH_O + (g + 1) * SB]
                nc.tensor.matmul(psB[:], xg, d8[:, g, HH:HID], start=False, stop=False)
            nc.tensor.matmul(
                psB[:], ones, brow[0:1, B1_O + HH : B1_O + HID],
                start=False, stop=True,
            )

            # ---- tail-lag corrections into psO [10, 64] ----
            psO = pp.tile([NCLS, SB], F32, tag="psO", bufs=1)
            for i in range(Kc - Km):
                nc.tensor.matmul(
                    psO[:],
                    f16b[:, GT_O + i * NCLS : GT_O + (i + 1) * NCLS],
                    xc[:, i, :],
                    start=(i == 0), stop=False,
                )

            # ---- evacuate phase 1: tanh -> yt (one ACT per bank) ----
            nc.scalar.activation(yt[:, 0:HH], psA[:], ACT.Tanh)
            nc.scalar.activation(yt[:, HH:HID], psB[:], ACT.Tanh)

            # ---- phase 2 with interleaved PE transposes of z1 tiles ----
            psC = pp.tile([64, HH], F32, tag="pA", bufs=1, name="psC")
            psD = pp.tile([64, HH], F32, tag="pB", bufs=1, name="psD")

            def emit_T(zt, src_yt, k, nm):
                pt = pp.tile([128, SB], F16, tag="pt", bufs=4, name=nm)
                nc.tensor.transpose(
                    pt[:], src_yt[:, k * 128 : (k + 1) * 128],
                    f16b[0:64, ID_O : ID_O + 64],
                )
                nc.vector.tensor_copy(zt[:, k, :], pt[:])

            for k in range(4):
                emit_T(z1t, yt, k, f"pt{k}")
            TAIL2 = NT - 2
            for k in range(TAIL2):
                if k + 4 < NT:
                    emit_T(z1t, yt, k + 4, f"pt{k + 4}")
                zk = z1t[:, k, :]
                nc.tensor.matmul(
                    psC[:], zk, w2[:, k, 0:HH],
                    start=(k == 0), stop=False,
                )
                nc.tensor.matmul(
                    psD[:], zk, w2[:, k, HH:HID],
                    start=(k == 0), stop=False,
                )
                nc.tensor.matmul(
                    psG[:], zk, f16b[:, G1_O + k * NCLS : G1_O + (k + 1) * NCLS],
                    start=False, stop=False,
                )
            for k in range(TAIL2, NT):
                zk = z1t[:, k, :]
                nc.tensor.matmul(psC[:], zk, w2[:, k, 0:HH], start=False, stop=False)
                nc.tensor.matmul(
                    psG[:], zk, f16b[:, G1_O + k * NCLS : G1_O + (k + 1) * NCLS],
                    start=False, stop=(k == NT - 1),
                )
            nc.tensor.matmul(
                psC[:], ones, brow[0:1, B2_O : B2_O + HH],
                start=False, stop=True,
            )
            for k in range(TAIL2, NT):
                zk = z1t[:, k, :]
                nc.tensor.matmul(psD[:], zk, w2[:, k, HH:HID], start=False, stop=False)
            nc.tensor.matmul(
                psD[:], ones, brow[0:1, B2_O + HH : B2_O + HID],
                start=False, stop=True,
            )
            

            # ---- evacuate phase 2: tanh(x/e2) -> yt2 ----
            nc.scalar.activation(yt2[:, 0:HH], psC[:], ACT.Tanh, scale=1.0 / E2)
            nc.scalar.activation(yt2[:, HH:HID], psD[:], ACT.Tanh, scale=1.0 / E2)

            # corr columns to fp16 while the out stage runs
            nc.vector.tensor_copy(corrall[:], psG[:])
            # b3 via K=1 matmul (independent of z2 - keep off the tail)
            nc.tensor.matmul(
                psO[:],
                brow[0:1, B3R_O : B3R_O + NCLS],
                ones,
                start=False, stop=False,
            )
            # transpose corr [64,10] -> [10,64] now (psG stopped long ago)
            ptC = pp.tile([128, SB], F16, tag="pt", bufs=4, name="ptC")
            nc.tensor.transpose(
                ptC[0:NCLS, :], corrall[:], f16b[0:64, ID_O : ID_O + 64]
            )
            nc.vector.tensor_copy(ptCsb[:], ptC[0:NCLS, :])

            # ---- out stage: psO += W3 @ z2, transposes interleaved ----
            emit_T(z2t, yt2, 0, "qt0")
            emit_T(z2t, yt2, 1, "qt1")
            for k in range(NT):
                if k + 2 < NT:
                    emit_T(z2t, yt2, k + 2, f"qt{k + 2}")
                nc.tensor.matmul(
                    psO[:],
                    f16b[:, W3_O + k * NCLS : W3_O + (k + 1) * NCLS],
                    z2t[:, k, :],
                    start=False, stop=(k == NT - 1),
                )
            # ot = ptCsb * (1/e_corr) + psO
            nc.vector.scalar_tensor_tensor(
                ot[:], ptCsb[:], 1.0 / ECORR, psO[:],
                mybir.AluOpType.mult, mybir.AluOpType.add,
            )
            nc.sync.dma_start(outd[:], ot[:])

    nc.compile()
    return nc


def _prep_weights(A, B, bias, W1, b1, W2, b2, W3, b3):
    """Host fp64 weight-only precompute (c1/c2 calibrated on synthetic
    gaussian x matching the spec'd input distribution, never the real x)."""
    B64 = B.astype(np.float64)
    W164 = W1.astype(np.float64)
    A64 = A.astype(np.float64)
    b64 = bias.astype(np.float64)
    W264 = W2.astype(np.float64)
    W364 = W3.astype(np.float64)

    Ds, M = [], A64.copy()
    for g in range(Kc):
        Ds.append(W164 @ M)
        M = B64 @ M
    Dsum = W164 @ np.linalg.solve(np.eye(HID) - B64, A64)
    b1f = b1.astype(np.float64) - Dsum @ b64

    rng = np.random.default_rng(12345)
    xcal = rng.standard_normal((Kc, 256, IN))
    Ycal = sum(xcal[g] @ Ds[g].T for g in range(Kc))
    c1 = float((1 - np.tanh(Ycal + b1f) ** 2).mean())
    y2cal = np.tanh(Ycal + b1f) @ W264.T + b2.astype(np.float64)
    c2 = float((1 - np.tanh(y2cal) ** 2).mean())

    D8 = np.empty((IN, Km, HID), NPE3)
    lagE, e_lag = [], []
    for g in range(Km):
        m = np.abs(Ds[g]).max()
        e = 2.0 ** np.clip(np.floor(np.log2(8.0 / m)), 0, 6)
        Dq = (Ds[g].T * e).astype(NPE3)
        D8[:, g, :] = Dq
        e_lag.append(e)
        lagE.append(e * Ds[g].T - Dq.astype(np.float64))

    mW2 = np.abs(W264).max()
    e2 = 2.0 ** np.floor(np.log2(8.0 / mW2))
    W2q = (W264.T * e2).astype(NPE3)              # [k, m]
    E2m = W264.T - W2q.astype(np.float64) / e2
    W2P = np.empty((IN, NT, HID), NPE3)
    for k in range(NT):
        W2P[:, k, :] = W2q[k * 128 : (k + 1) * 128, :]

    CWm = c1 * c2 * (W264.T @ W364.T)
    Gq = [lagE[g] @ CWm for g in range(Km)]       # [IN, 10] at xq scale
    G1 = c2 * (E2m @ W364.T)                      # [k, 10] applied to z1
    Gt = [Ds[g].T @ CWm for g in range(Km, Kc)]   # [IN, 10] at true x scale

    gmax = max(max(np.abs(g_).max() for g_ in Gq), np.abs(G1).max())
    e_corr = 2.0 ** np.floor(np.log2(8.0 / gmax))

    brow = np.zeros((1, BROWW), np.float16)
    brow[0, B1_O : B1_O + HID] = b1f.astype(np.float16)
    brow[0, B2_O : B2_O + HID] = (b2.astype(np.float64) * e2).astype(np.float16)
    brow[0, ON_O : ON_O + SB] = 1.0
    brow[0, B3R_O : B3R_O + NCLS] = b3.astype(np.float16)

    f16c = np.zeros((128, F16W), np.float16)
    W3T = W364.T.astype(np.float16)               # [HID, 10]
    for k in range(NT):
        f16c[:, W3_O + k * NCLS : W3_O + (k + 1) * NCLS] = (
            W3T[k * 128 : (k + 1) * 128, :]
        )
    for i in range(Kc - Km):
        f16c[:, GT_O + i * NCLS : GT_O + (i + 1) * NCLS] = Gt[i].astype(np.float16)
    for g in range(Km):
        f16c[:, GQ_O + g * NCLS : GQ_O + (g + 1) * NCLS] = (
            (Gq[g] * e_corr).astype(np.float16)
        )
    for k in range(NT):
        f16c[:, G1_O + k * NCLS : G1_O + (k + 1) * NCLS] = (
            (G1[k * 128 : (k + 1) * 128, :] * e_corr).astype(np.float16)
        )
    f16c[0:64, ID_O : ID_O + 64] = np.eye(64, dtype=np.float16)

    return {
        "e_lag": e_lag, "e2": e2, "e_corr": e_corr,
        "D8": D8, "W2P": W2P, "brow": brow, "f16c": f16c,
        "c1": c1, "c2": c2,
    }


def _prep_inputs(x, wp, ncores=NCORES):
    in_maps = []
    for c in range(ncores):
        bsl = slice(c * SB, (c + 1) * SB)
        f16b = wp["f16c"].copy()
        for g in range(Km):
            f16b[:, XH_O + g * SB : XH_O + (g + 1) * SB] = (
                x[T - 1 - g, bsl, :].T / wp["e_lag"][g]
            ).astype(np.float16)
        XC = np.empty((IN, (Kc - Km) * SB), NPE4)
        for i, g in enumerate(range(Km, Kc)):
            XC[:, i * SB : (i + 1) * SB] = x[T - 1 - g, bsl, :].T.astype(NPE4)
        in_maps.append(
            {
                "F16B": f16b,
                "BROW": wp["brow"],
                "D8": wp["D8"],
                "W2P": wp["W2P"],
                "XC": XC,
            }
        )
    return in_maps


def kernel(x, A, B, bias, W1, b1, W2, b2, W3, b3, _trace=False):
    wp = _prep_weights(A, B, bias, W1, b1, W2, b2, W3, b3)
    assert wp["e2"] == E2, "activation scale 1/e2 hardcoded in program"
    assert wp["e_corr"] == ECORR, "1/e_corr hardcoded in program"
    if "nc" not in _PROGRAM_CACHE:
        _PROGRAM_CACHE["nc"] = _build_program()
    nc = _PROGRAM_CACHE["nc"]
    in_maps = _prep_inputs(x, wp)
    res = run_bass_kernel_spmd(nc, in_maps, list(range(NCORES)), trace=_trace)
    _PROGRAM_CACHE["last_result"] = res
    out = np.empty((BATCH, NCLS), np.float32)
    for c in range(NCORES):
        out[c * SB : (c + 1) * SB, :] = res.results[c]["out"].T
    return out
